# revision 33
# baseline (speedup 1.0000x reference)
"""VQ-codebook 3x3 conv (nn_CConv) on 8 Trainium2 NeuronCores.

Sharding: data-parallel over the batch (16 images -> 2 per core); the small
codebook-derived weight matrix / bias are replicated to every core.
Host-side work is prep only: batch split, zero-padding of x to 114x114 with
an f16 cast, the codebook row gather plus per-(out,in) scale (scales
round-tripped through f16 = dequant emulation, * cut) folded into a k-major
f16 weight matrix, and the f16 -> f32 upcast of the output.  All conv
arithmetic (7.4 GFLOP/core) runs on device.

Per-core device program (one NEFF, SPMD over 8 cores):
  - each padded image lives whole in SBUF ([128, 114*114] f16, loaded as
    contiguous row-chunk DMAs on the sync queue; the first chunk is small
    so the PE starts early).  Weights/bias ride the scalar HWDGE queue,
    which is idle until the first output DMA.
  - conv: per PSUM tile of 4 output rows (N = 4*112 = 448 <= 512 fp32
    PSUM bank), 9 accumulating PE matmuls, one per tap (dh, dw); the
    moving operand is the 2D strided view xpad[:, h0+dh : h0+dh+4,
    dw : dw+112], which the PE streams at 1 column/cycle with no
    rollover penalty, so no junk columns are ever computed.  Weights are
    f16 stationary [128(in), 128(out)] blocks; LDWEIGHTS hides in the
    PE's background weight buffer.  All 513 matmuls run back-to-back
    with zero gaps at 2.4 GHz (190 ns per 448-column MM = the floor).
  - seven dummy warm-up matmuls fill the otherwise-idle PE window between
    program start and the first data landing (~4.5 us), burning off the
    HAM clock-gate's ~3.4 us cold window so the real stream runs at
    2.4 GHz from its first instruction.  They are sized to end just as
    the weight/input DMAs land, so they never push the real work back.
  - PSUM is evacuated with a fused per-partition bias add on the vector
    engine straight to f16; output DMAs ride the scalar HWDGE queue in
    row-chunks as soon as their tiles are evacuated.  The last image ends
    with two 2-row PSUM tiles + a 2-row output chunk to keep the final
    evac->DMA tail short.
"""
import sys
import types
from contextlib import ExitStack

import numpy as np

import concourse.tile as tile
from concourse import bacc, mybir


def _ensure_axon_hooks_module():
    """This image's antenv package lacks axon_hooks; bass_utils imports it
    when tracing is requested (e.g. BASS_TRACE=1). Provide a no-op shim."""
    try:
        import antenv

        if "antenv.axon_hooks" not in sys.modules and not hasattr(
            antenv, "axon_hooks"
        ):
            mod = types.ModuleType("antenv.axon_hooks")
            holder = [None]
            mod.set_axon_ntff_profile_hook = lambda h: holder.__setitem__(0, h)
            mod.get_axon_ntff_profile_hook = lambda: holder[0]
            antenv.axon_hooks = mod
            sys.modules["antenv.axon_hooks"] = mod
    except Exception:
        pass


_ensure_axon_hooks_module()

from concourse import bass_utils  # noqa: E402

P = 128
H = W = 112
WP = 114
NPIX = WP * WP          # padded image pixels per channel (114*114 = 12996)
NO = H * W              # output pixels per channel (112*112 = 12544)
IMGS = 2
N_CORES = 8

f32 = mybir.dt.float32
f16 = mybir.dt.float16

IN_CHUNKS = [(0, 8), (8, 32), (32, 60), (60, 88), (88, 114)]  # padded input rows
IN_CHUNKS_LATER = [(0, 57), (57, 114)]
OUT_CHUNKS = [(0, 112)]                                    # output rows
OUT_CHUNKS_LAST = [(0, 28), (28, 56), (56, 84), (84, 104), (104, 110), (110, 112)]

_CACHE = {}


def _row_tiles(last):
    # 4 output rows x 112 cols = 448 <= 512 PSUM bank; on the last image
    # end with two 2-row tiles so the final PSUM evacuation (and the
    # output DMA chained behind it) is short.  (Measured: tapering
    # further to 1-row tiles/chunks is a net loss -- each extra
    # dma_start adds ~0.7us of serialized descriptor-gen + completion
    # receipt, more than the smaller transfer saves.)
    if last:
        return [4] * 27 + [2, 2]
    return [4] * 28


def _build():
    nc = bacc.Bacc("TRN2", target_bir_lowering=False, debug=False)

    x_t = nc.dram_tensor("x", [IMGS, P, NPIX], f16, kind="ExternalInput")
    bias_t = nc.dram_tensor("bias", [P, 1], f32, kind="ExternalInput")
    wmm_t = nc.dram_tensor("wmm", [P, P * 9], f16, kind="ExternalInput")
    out_t = nc.dram_tensor("out", [IMGS, P, NO], f16, kind="ExternalOutput")

    with tile.TileContext(nc) as tc, ExitStack() as ctx:
        wb = ctx.enter_context(tc.tile_pool(name="wb", bufs=1))
        xp = ctx.enter_context(tc.tile_pool(name="xp", bufs=2))
        op = ctx.enter_context(tc.tile_pool(name="op", bufs=2))
        ps = ctx.enter_context(tc.tile_pool(name="ps", bufs=8, space="PSUM"))

        # the PE is otherwise idle from program start until the weight +
        # first-input DMAs land (~4.5us): fill that window with dummy
        # matmuls so the HAM clock-gate's ~3.4us cold window burns off
        # before the real stream starts, which then runs at 2.4 GHz from
        # its first instruction.  Sized to end just before the data lands
        # so they never push the real matmuls back.
        wrm = wb.tile([P, 512], f16, tag="warm")
        nc.gpsimd.memset(wrm[:], 0.0)
        pw = ps.tile([P, 512], f32, tag="pst")
        for _ in range(8):
            nc.tensor.matmul(pw[:], wrm[:, :P], wrm[:], start=True, stop=True)

        # ---- weights: host-built w_mm[i, k, o] (k-major f16 taps) ----
        # ride the scalar HWDGE queue (idle until the first output DMA)
        # so the input chunks on the sync queue start immediately.
        # (Measured: splitting this DMA to start LDWEIGHTS earlier is a
        # wash -- each extra dma_start costs ~0.7us of descriptor
        # generation on the queue and risks mid-stream weight stalls.)
        w_mm = wb.tile([P, 9 * P], f16, tag="w_mm")
        nc.scalar.dma_start(w_mm[:], wmm_t.ap())
        bias_s = wb.tile([P, 1], f32, tag="bias")
        nc.scalar.dma_start(bias_s[:], bias_t.ap())
        w_k_view = w_mm[:].rearrange("p (k o) -> p k o", o=P)

        # ---- conv, whole image resident in SBUF ----
        for img in range(IMGS):
            xpad = xp.tile([P, NPIX], f16, tag="xpad")
            for r0, r1 in (IN_CHUNKS if img == 0 else IN_CHUNKS_LATER):
                nc.sync.dma_start(
                    xpad[:, r0 * WP:r1 * WP], x_t.ap()[img, :, r0 * WP:r1 * WP]
                )
            xpad3 = xpad[:].rearrange("p (r c) -> p r c", c=WP)

            oimg = op.tile([P, NO], f16, tag="oimg")
            ochunks = list(OUT_CHUNKS_LAST if img == IMGS - 1 else OUT_CHUNKS)
            h0 = 0
            for nr in _row_tiles(img == IMGS - 1):
                n = nr * W
                pst = ps.tile([P, 512], f32, tag="pst")
                for k in range(9):
                    dh, dw = divmod(k, 3)
                    nc.tensor.matmul(
                        pst[:, :n],
                        w_k_view[:, k, :],
                        xpad3[:, h0 + dh:h0 + dh + nr, dw:dw + W],
                        start=(k == 0),
                        stop=(k == 8),
                    )
                nc.vector.tensor_scalar_add(
                    oimg[:, h0 * W:(h0 + nr) * W], pst[:, :n], bias_s[:, 0:1]
                )
                h0 += nr
                while ochunks and h0 >= ochunks[0][1]:
                    r0, r1 = ochunks.pop(0)
                    nc.scalar.dma_start(
                        out_t.ap()[img, :, r0 * W:r1 * W],
                        oimg[:, r0 * W:r1 * W],
                    )
            assert not ochunks and h0 == H

    nc.compile()
    return nc


def _make_in_maps(inputs):
    x = np.asarray(inputs["x"], dtype=np.float32)
    nimg = x.shape[0]
    xpad = np.zeros((nimg, P, WP, WP), dtype=np.float16)
    xpad[:, :, 1:1 + H, 1:1 + W] = x.astype(np.float16)
    xpad = xpad.reshape(nimg, P, NPIX)

    cent = np.asarray(inputs["centroids"], dtype=np.float32).reshape(512, 9)
    idxT = np.asarray(inputs["idx"]).reshape(P, P).T          # [i, o]
    # fp16 round-trip of scales (dequant emulation), * cut
    scalesT = (
        np.asarray(inputs["scales"], dtype=np.float32).reshape(P, P).T
        .astype(np.float16).astype(np.float32)
    )
    cutT = np.asarray(inputs["cut"], dtype=np.float32).reshape(P, P).T
    bias = np.ascontiguousarray(
        np.asarray(inputs["bias"], dtype=np.float32).reshape(P, 1)
    )
    # w_mm[i, k, o] = w_raw[i, o, k] * scales_q[i, o] * cut[i, o], f16 taps
    wraw = cent[idxT].reshape(P, P, 9)                        # [i, o, k]
    wmm = np.ascontiguousarray(
        (wraw * (scalesT * cutT)[:, :, None])
        .transpose(0, 2, 1).reshape(P, P * 9).astype(np.float16)
    )

    base = {"bias": bias, "wmm": wmm}
    maps = []
    for c in range(N_CORES):
        m = dict(base)
        m["x"] = np.ascontiguousarray(xpad[IMGS * c:IMGS * (c + 1)])
        maps.append(m)
    return maps


def _get_nc():
    if "nc" not in _CACHE:
        _CACHE["nc"] = _build()
    return _CACHE["nc"]


def _run(inputs, trace=False):
    nc = _get_nc()
    in_maps = _make_in_maps(inputs)
    res = bass_utils.run_bass_kernel_spmd(
        nc, in_maps, core_ids=list(range(N_CORES)), trace=trace
    )
    outp = np.concatenate(
        [res.results[c]["out"] for c in range(N_CORES)], axis=0
    )
    out = outp.reshape(-1, P, H, W).astype(np.float32)
    return np.ascontiguousarray(out), res


def kernel(**inputs) -> np.ndarray:
    out, _ = _run(inputs, trace=False)
    return out
